# revision 3
# baseline (speedup 1.0000x reference)
"""Trainium2 Bass kernel for an 8-head MultiHeadAttention (B=2, S=4096, H=512).

Sharding: 8 NeuronCores, each takes (one batch, two heads):
    core c -> batch b = c // 4, heads {2*(c%4), 2*(c%4)+1}.

Optimized pipeline (vs. the 358us baseline):
  - Projections run in bf16 (x and W pre-cast on host): halves the input
    DMA (4MB/core) and keeps PE at 1 cycle/row.  Wave order k -> q -> v so
    scores can start as soon as q's first block is evicted.
  - The attention inner loop is software-pipelined with a lookahead of
    LOOK kc-chunks so the PE never waits on the softmax exp:
        scores(kc) -> exp(kc) [split across engines] -> attn@v(kc-LOOK)
  - The exp of each [128, 2*512] score tile is split across THREE engines
    working concurrently on disjoint column ranges: Act (true Exp), DVE
    and GpSimd (Schraudolph bit-trick exp: bf16 bits = int16(s*A + B),
    +-3.3% multiplicative, validated ~1.3e-2 scale-relative absmax).
  - Normalization: denominator row (from the ones-column in v) is copied
    to f32r, broadcast across 64 partitions by a K=1 ones-matmul,
    reciprocal_approx_fast + multiply on DVE, bf16 output (host upcasts).
"""

import sys

sys.path.insert(0, "/opt/trn_rl_repo")

import ml_dtypes
import numpy as np

import concourse.bass as bass  # noqa: E402
import concourse.tile as tile  # noqa: E402
from concourse import bacc, mybir  # noqa: E402
from concourse.bass_utils import run_bass_kernel_spmd  # noqa: E402

B, S, H = 2, 4096, 512
NH, HD = 8, 64
NCORES = 8
HPC = 2  # heads per core
DPC = HPC * HD  # head dims per core = 128
P = 128  # partitions
QB = 512  # query block (matmul free dim)
KC = 128  # key chunk (contraction tile)
KF = H // P  # feature chunks for projections = 4
NKC = S // KC  # 32
NQB = S // QB  # 8
VPAD = 80  # padded per-(kc,h) v row (64 v + ones + align padding)
LOOK = 2  # attn@v lookahead (kc chunks) for PE pipelining

# exp column split of the flat [128, 1024] score tile (Act | DVE).
# GpSimd cannot read PSUM, so only these two engines can evict scores.
ACT_END = 544
DVE_END = 1024

# Schraudolph constants: bf16(exp(s/8)) bits ~= int16(s*EXP_A + EXP_B)
SCALE = 1.0 / np.sqrt(HD)
EXP_A = float(128.0 * np.log2(np.e) * SCALE)
EXP_B = float(128.0 * (127.0 - 0.0436))

f32 = mybir.dt.float32
f32r = mybir.dt.float32r
bf16 = mybir.dt.bfloat16
i16 = mybir.dt.int16
_np_bf16 = ml_dtypes.bfloat16


def _emit_kernel(ctx, tc, outT, xT, wq, wk, wv, bias3, ones, onescol):
    nc = tc.nc

    const = ctx.enter_context(tc.tile_pool(name="const", bufs=1))

    # ---- weights/constants first (small), then x: PE unblocks early ----
    wq_sb = const.tile([P, KF, DPC], bf16)
    wk_sb = const.tile([P, KF, DPC], bf16)
    wv_sb = const.tile([P, KF, DPC], bf16)
    for w_sb, w in ((wk_sb, wk), (wq_sb, wq), (wv_sb, wv)):
        nc.sync.dma_start(
            out=w_sb[:], in_=w.rearrange("(kf p) m -> p kf m", p=P)
        )
    # biases [3, 128] -> sbuf [128, 3] (partition = output dim; q, k, v)
    bias_sb = const.tile([P, 3], f32)
    nc.sync.dma_start(out=bias_sb[:], in_=bias3.rearrange("a m -> m a"))
    ones_sb = const.tile([1, P], f32r)
    nc.sync.dma_start(out=ones_sb[:], in_=ones[:])

    # xT [H, S] -> sbuf [128, KF, S] (partition = feature % 128);
    # half-chunks so the first wave's matmuls unblock sooner
    xT_sb = const.tile([P, KF, S], bf16)
    for kf in range(KF):
        for hh in range(2):
            nc.sync.dma_start(
                out=xT_sb[:, kf, hh * (S // 2) : (hh + 1) * (S // 2)],
                in_=xT[kf * P : (kf + 1) * P, hh * (S // 2) : (hh + 1) * (S // 2)],
            )

    # ---- projections: q/k/v in T layout, bf16 matmuls, bf16 evictions ----
    qkT_sb = const.tile([P, 2, S], bf16)
    vT_sb = const.tile([P, S], bf16)
    # v natural + ones column: vp_sb[p, kc, h, :64] = v, [..., 64] = 1
    vp_sb = const.tile([P, NKC, HPC, VPAD], bf16)
    nc.sync.dma_start(out=vp_sb[:, :, :, HD : HD + 1], in_=onescol[:])

    with tc.tile_pool(name="proj_psum", bufs=8, space="PSUM") as pp:
        with nc.named_scope("proj"):
            # wave order: k, q (so scores can start), then v
            for proj, w_sb in ((1, wk_sb), (0, wq_sb), (2, wv_sb)):
                pss = [
                    pp.tile([P, QB], f32, tag="ps", name=f"pj{proj}_{sb}")
                    for sb in range(S // QB)
                ]
                # kf-outer: the first 8 matmuls need only xT chunk 0
                for kf in range(KF):
                    for sb in range(S // QB):
                        nc.tensor.matmul(
                            pss[sb][:],
                            lhsT=w_sb[:, kf, :],
                            rhs=xT_sb[:, kf, sb * QB : (sb + 1) * QB],
                            start=(kf == 0),
                            stop=(kf == KF - 1),
                        )
                for sb in range(S // QB):
                    dst = (
                        vT_sb[:, sb * QB : (sb + 1) * QB]
                        if proj == 2
                        else qkT_sb[:, proj, sb * QB : (sb + 1) * QB]
                    )
                    # psum -> sbuf eviction, fused bias add, bf16 out
                    with nc.allow_low_precision(reason="bf16 attention"):
                        nc.vector.tensor_scalar_add(
                            dst, pss[sb][:], bias_sb[:, proj : proj + 1]
                        )
                if proj == 2:
                    # v: T layout -> natural via hardware DMA transpose
                    # (X-bar, bf16), one per head: in [64, S] -> out
                    # [128, NKC, 64].  All input DMAs have drained by now;
                    # the transposes overlap the first scores/exp waves.
                    # (Finer-grained splits that overlap the eviction
                    # stream hard-crash the device - do not pipeline these.)
                    for h in range(HPC):
                        nc.sync.dma_start_transpose(
                            out=vp_sb[:, :, h, 0:HD],
                            in_=vT_sb[h * HD : (h + 1) * HD, :],
                        )

    # ---- attention ----
    sc_pool = ctx.enter_context(tc.tile_pool(name="sc", bufs=2, space="PSUM"))
    rb_pool = ctx.enter_context(tc.tile_pool(name="rb", bufs=2, space="PSUM"))
    ot_pool = ctx.enter_context(tc.tile_pool(name="ot", bufs=1, space="PSUM"))
    ex_pool = ctx.enter_context(tc.tile_pool(name="ex", bufs=4))
    rc_pool = ctx.enter_context(tc.tile_pool(name="rc", bufs=4))
    res_pool = ctx.enter_context(tc.tile_pool(name="res", bufs=4))

    with nc.named_scope("attn"):
        for qb in range(NQB):
            q0, q1 = qb * QB, (qb + 1) * QB
            oT = [
                ot_pool.tile([HD + 1, QB], f32, tag=f"oT{h}", name=f"oT{qb}_{h}")
                for h in range(HPC)
            ]
            ex_tiles = {}

            def attnv(kc):
                for h in range(HPC):
                    nc.tensor.matmul(
                        oT[h][:],
                        lhsT=vp_sb[:, kc, h, 0 : HD + 1],
                        rhs=ex_tiles[kc][:, h, :],
                        start=(kc == 0),
                        stop=(kc == NKC - 1),
                    )

            for kc in range(NKC):
                sc = sc_pool.tile([P, HPC, QB], f32, tag="sc")
                for h in range(HPC):
                    # scoresT[k, q] for head h; K = 64, rows 64h..64h+63
                    nc.tensor.matmul(
                        sc[:, h, :],
                        lhsT=qkT_sb[
                            h * HD : (h + 1) * HD, 1, kc * KC : (kc + 1) * KC
                        ],
                        rhs=qkT_sb[h * HD : (h + 1) * HD, 0, q0:q1],
                        start=True,
                        stop=True,
                        tile_position=(h * HD, 0),
                    )
                ex = ex_pool.tile([P, HPC, QB], bf16, tag="ex")
                ex_tiles[kc] = ex
                scf = sc.rearrange("p a b -> p (a b)")
                exf = ex.rearrange("p a b -> p (a b)")
                # exp split across three engines on disjoint column ranges
                nc.scalar.activation(
                    exf[:, 0:ACT_END],
                    scf[:, 0:ACT_END],
                    mybir.ActivationFunctionType.Exp,
                    scale=SCALE,
                )
                with nc.allow_low_precision(reason="schraudolph exp"):
                    nc.vector.tensor_scalar(
                        exf[:, ACT_END:DVE_END].bitcast(i16),
                        scf[:, ACT_END:DVE_END],
                        EXP_A,
                        EXP_B,
                        mybir.AluOpType.mult,
                        mybir.AluOpType.add,
                    )
                    if DVE_END < HPC * QB:
                        nc.gpsimd.tensor_scalar(
                            exf[:, DVE_END:].bitcast(i16),
                            scf[:, DVE_END:],
                            EXP_A,
                            EXP_B,
                            mybir.AluOpType.mult,
                            mybir.AluOpType.add,
                        )
                if kc >= LOOK:
                    attnv(kc - LOOK)
            for kc in range(NKC - LOOK, NKC):
                attnv(kc)

            for h in range(HPC):
                # denominator row -> f32r sbuf (matmul rhs must be sbuf)
                srow = rc_pool.tile([1, QB], f32r, tag="srow", name=f"sr{qb}_{h}")
                with nc.allow_low_precision(reason="f32r sums, 2^-12 rel"):
                    nc.vector.tensor_copy(srow[:], oT[h][HD : HD + 1, :])
                # K=1 ones-matmul broadcasts the denominators to 64 rows
                rb = rb_pool.tile([HD, QB], f32, tag="rb", name=f"rb{qb}_{h}")
                nc.tensor.matmul(
                    rb[:],
                    lhsT=ones_sb[:, :HD],
                    rhs=srow[:],
                    start=True,
                    stop=True,
                )
                rcb = res_pool.tile([HD, QB], f32, tag="rcb", name=f"rcb{qb}_{h}")
                nc.vector.reciprocal_approx_fast(out=rcb[:], in_=rb[:])
                res = res_pool.tile([HD, QB], bf16, tag="res")
                with nc.allow_low_precision(reason="bf16 output"):
                    nc.vector.tensor_mul(res[:], oT[h][:HD, :], rcb[:])
                nc.sync.dma_start(
                    out=outT[h * HD : (h + 1) * HD, q0:q1], in_=res[:]
                )


def build_nc():
    from contextlib import ExitStack

    nc = bacc.Bacc(
        "TRN2",
        target_bir_lowering=False,
        debug=False,
        num_devices=NCORES,
    )
    xT = nc.dram_tensor("xT", [H, S], bf16, kind="ExternalInput").ap()
    wq = nc.dram_tensor("wq", [H, DPC], bf16, kind="ExternalInput").ap()
    wk = nc.dram_tensor("wk", [H, DPC], bf16, kind="ExternalInput").ap()
    wv = nc.dram_tensor("wv", [H, DPC], bf16, kind="ExternalInput").ap()
    bias3 = nc.dram_tensor("bias3", [3, DPC], f32, kind="ExternalInput").ap()
    ones = nc.dram_tensor("ones", [1, P], f32r, kind="ExternalInput").ap()
    onescol = nc.dram_tensor(
        "onescol", [P, NKC * HPC], bf16, kind="ExternalInput"
    ).ap()
    outT = nc.dram_tensor("outT", [DPC, S], bf16, kind="ExternalOutput").ap()
    with tile.TileContext(nc) as tc, ExitStack() as ctx:
        _emit_kernel(ctx, tc, outT, xT, wq, wk, wv, bias3, ones, onescol)
    nc.compile()
    return nc


_NC_CACHE = None


def _get_nc():
    global _NC_CACHE
    if _NC_CACHE is None:
        _NC_CACHE = build_nc()
    return _NC_CACHE


def _shard_inputs(x, Wq, bq, Wk, bk, Wv, bv):
    """Build per-core input maps (host does layout only: transpose/slice)."""
    x = np.ascontiguousarray(np.asarray(x, dtype=np.float32))
    in_maps = []
    xT_by_batch = [np.ascontiguousarray(x[b].T).astype(_np_bf16) for b in range(B)]
    for c in range(NCORES):
        b, p = c // (NCORES // B), c % (NCORES // B)
        cols = slice(p * DPC, (p + 1) * DPC)
        in_maps.append(
            {
                "xT": xT_by_batch[b],
                "wq": np.asarray(Wq, np.float32)[:, cols].astype(_np_bf16),
                "wk": np.asarray(Wk, np.float32)[:, cols].astype(_np_bf16),
                "wv": np.asarray(Wv, np.float32)[:, cols].astype(_np_bf16),
                "bias3": np.stack(
                    [
                        np.asarray(bq, np.float32)[cols],
                        np.asarray(bk, np.float32)[cols],
                        np.asarray(bv, np.float32)[cols],
                    ]
                ),
                "ones": np.ones((1, P), dtype=np.float32),
                "onescol": np.ones((P, NKC * HPC), dtype=_np_bf16),
            }
        )
    return in_maps


def _assemble(results):
    out = np.empty((B, S, H), dtype=np.float32)
    for c in range(NCORES):
        b, p = c // (NCORES // B), c % (NCORES // B)
        outT = results[c]["outT"]  # [128, S] bf16
        out[b, :, p * DPC : (p + 1) * DPC] = outT.astype(np.float32).T
    return out


def run(inputs, trace=False):
    nc = _get_nc()
    in_maps = _shard_inputs(**inputs)
    res = run_bass_kernel_spmd(nc, in_maps, list(range(NCORES)), trace=trace)
    return _assemble(res.results), res


def kernel(**inputs):
    out, _ = run(inputs)
    return out


# revision 5
# speedup vs baseline: 1.1622x; 1.1622x over previous
"""Trainium2 Bass kernel for an 8-head MultiHeadAttention (B=2, S=4096, H=512).

Sharding: 8 NeuronCores, each takes (one batch, two heads):
    core c -> batch b = c // 4, heads {2*(c%4), 2*(c%4)+1}.

Optimized pipeline (vs. the 358us baseline):
  - Projections run in bf16 (x and W pre-cast on host): halves the input
    DMA (4MB/core) and keeps PE at 1 cycle/row.  Wave order k -> q -> v so
    scores can start as soon as q's first block is evicted.
  - The attention inner loop is software-pipelined with a lookahead of
    LOOK kc-chunks so the PE never waits on the softmax exp:
        scores(kc) -> exp(kc) [split across engines] -> attn@v(kc-LOOK)
  - The exp of each [128, 2*512] score tile is split across THREE engines
    working concurrently on disjoint column ranges: Act (true Exp), DVE
    and GpSimd (Schraudolph bit-trick exp: bf16 bits = int16(s*A + B),
    +-3.3% multiplicative, validated ~1.3e-2 scale-relative absmax).
  - Normalization: denominator row (from the ones-column in v) is copied
    to f32r, broadcast across 64 partitions by a K=1 ones-matmul,
    reciprocal_approx_fast + multiply on DVE, bf16 output (host upcasts).
"""

import sys

sys.path.insert(0, "/opt/trn_rl_repo")

import ml_dtypes
import numpy as np

import concourse.bass as bass  # noqa: E402
import concourse.tile as tile  # noqa: E402
from concourse import bacc, mybir  # noqa: E402
from concourse.bass_utils import run_bass_kernel_spmd  # noqa: E402

B, S, H = 2, 4096, 512
NH, HD = 8, 64
NCORES = 8
HPC = 2  # heads per core
DPC = HPC * HD  # head dims per core = 128
P = 128  # partitions
QB = 512  # query block (matmul free dim)
KC = 128  # key chunk (contraction tile)
KF = H // P  # feature chunks for projections = 4
NKC = S // KC  # 32
NQB = S // QB  # 8
VPAD = 80  # padded per-(kc,h) v row (64 v + ones + align padding)
LOOK = 2  # attn@v lookahead (kc chunks) for PE pipelining

# Per-head exp engines: Act owns head 0, DVE (Schraudolph) owns head 1.
# GpSimd cannot read PSUM, so only these two engines can evict scores.
LOOK_H = (2, 3)  # attn@v lookahead per head (h1 later: DVE exp is slower)

# Schraudolph constants: bf16(exp(s/8)) bits ~= int16(s*EXP_A + EXP_B)
SCALE = 1.0 / np.sqrt(HD)
EXP_A = float(128.0 * np.log2(np.e) * SCALE)
EXP_B = float(128.0 * (127.0 - 0.0436))

f32 = mybir.dt.float32
f32r = mybir.dt.float32r
bf16 = mybir.dt.bfloat16
i16 = mybir.dt.int16
_np_bf16 = ml_dtypes.bfloat16


def _emit_kernel(ctx, tc, outT, xT, wq, wk, wv, bias3, ones, onescol):
    nc = tc.nc

    const = ctx.enter_context(tc.tile_pool(name="const", bufs=1))

    # ---- weights/constants first (small), then x: PE unblocks early ----
    wq_sb = const.tile([P, KF, DPC], bf16)
    wk_sb = const.tile([P, KF, DPC], bf16)
    wv_sb = const.tile([P, KF, DPC], bf16)
    for w_sb, w in ((wk_sb, wk), (wq_sb, wq), (wv_sb, wv)):
        nc.sync.dma_start(
            out=w_sb[:], in_=w.rearrange("(kf p) m -> p kf m", p=P)
        )
    # biases [3, 128] -> sbuf [128, 3] (partition = output dim; q, k, v)
    bias_sb = const.tile([P, 3], f32)
    nc.sync.dma_start(out=bias_sb[:], in_=bias3.rearrange("a m -> m a"))
    ones_sb = const.tile([1, P], f32r)
    nc.sync.dma_start(out=ones_sb[:], in_=ones[:])

    # xT [H, S] -> sbuf [128, KF, S] (partition = feature % 128);
    # half-chunks so the first wave's matmuls unblock sooner
    xT_sb = const.tile([P, KF, S], bf16)
    for kf in range(KF):
        for hh in range(2):
            nc.sync.dma_start(
                out=xT_sb[:, kf, hh * (S // 2) : (hh + 1) * (S // 2)],
                in_=xT[kf * P : (kf + 1) * P, hh * (S // 2) : (hh + 1) * (S // 2)],
            )

    # ---- projections: q/k/v in T layout, bf16 matmuls, bf16 evictions ----
    qkT_sb = const.tile([P, 2, S], bf16)
    vT_sb = const.tile([P, S], bf16)
    # v natural + ones column: vp_sb[p, kc, h, :64] = v, [..., 64] = 1
    vp_sb = const.tile([P, NKC, HPC, VPAD], bf16)
    nc.sync.dma_start(out=vp_sb[:, :, :, HD : HD + 1], in_=onescol[:])

    with tc.tile_pool(name="proj_psum", bufs=8, space="PSUM") as pp:
        with nc.named_scope("proj"):
            # wave order: k, q (so scores can start), then v
            for proj, w_sb in ((1, wk_sb), (0, wq_sb), (2, wv_sb)):
                pss = [
                    pp.tile([P, QB], f32, tag="ps", name=f"pj{proj}_{sb}")
                    for sb in range(S // QB)
                ]
                # kf-outer: the first 8 matmuls need only xT chunk 0
                for kf in range(KF):
                    for sb in range(S // QB):
                        nc.tensor.matmul(
                            pss[sb][:],
                            lhsT=w_sb[:, kf, :],
                            rhs=xT_sb[:, kf, sb * QB : (sb + 1) * QB],
                            start=(kf == 0),
                            stop=(kf == KF - 1),
                        )
                for sb in range(S // QB):
                    dst = (
                        vT_sb[:, sb * QB : (sb + 1) * QB]
                        if proj == 2
                        else qkT_sb[:, proj, sb * QB : (sb + 1) * QB]
                    )
                    # psum -> sbuf eviction, fused bias add, bf16 out
                    with nc.allow_low_precision(reason="bf16 attention"):
                        nc.vector.tensor_scalar_add(
                            dst, pss[sb][:], bias_sb[:, proj : proj + 1]
                        )
                if proj == 2:
                    # v: T layout -> natural via hardware DMA transpose
                    # (X-bar, bf16), one per head: in [64, S] -> out
                    # [128, NKC, 64].  All input DMAs have drained by now;
                    # the transposes overlap the first scores/exp waves.
                    # (Finer-grained splits that overlap the eviction
                    # stream hard-crash the device - do not pipeline these.)
                    for h in range(HPC):
                        nc.sync.dma_start_transpose(
                            out=vp_sb[:, :, h, 0:HD],
                            in_=vT_sb[h * HD : (h + 1) * HD, :],
                        )

    # ---- attention ----
    # PSUM budget (8 banks): sch 4 + oT 2 + rb 2
    sc_pool = ctx.enter_context(tc.tile_pool(name="sc", bufs=4, space="PSUM"))
    rb_pool = ctx.enter_context(tc.tile_pool(name="rb", bufs=2, space="PSUM"))
    ot_pool = ctx.enter_context(tc.tile_pool(name="ot", bufs=1, space="PSUM"))
    ex_pool = ctx.enter_context(tc.tile_pool(name="ex", bufs=5))
    rc_pool = ctx.enter_context(tc.tile_pool(name="rc", bufs=4))
    res_pool = ctx.enter_context(tc.tile_pool(name="res", bufs=4))

    with nc.named_scope("attn"):
        for qb in range(NQB):
            q0, q1 = qb * QB, (qb + 1) * QB
            oT = [
                ot_pool.tile([HD + 1, QB], f32, tag=f"oT{h}", name=f"oT{qb}_{h}")
                for h in range(HPC)
            ]
            ex_tiles = {}

            def attnv(kc, h):
                nc.tensor.matmul(
                    oT[h][:],
                    lhsT=vp_sb[:, kc, h, 0 : HD + 1],
                    rhs=ex_tiles[kc][:, h, :],
                    start=(kc == 0),
                    stop=(kc == NKC - 1),
                )

            for kc in range(NKC):
                # per-head 1-bank score tiles -> deeper recycle slack
                scs = [
                    sc_pool.tile([P, QB], f32, tag="sch", name=f"sc{qb}_{kc}_{h}")
                    for h in range(HPC)
                ]
                for h in range(HPC):
                    # scoresT[k, q] for head h; K = 64, rows 64h..64h+63
                    nc.tensor.matmul(
                        scs[h][:],
                        lhsT=qkT_sb[
                            h * HD : (h + 1) * HD, 1, kc * KC : (kc + 1) * KC
                        ],
                        rhs=qkT_sb[h * HD : (h + 1) * HD, 0, q0:q1],
                        start=True,
                        stop=True,
                        tile_position=(h * HD, 0),
                    )
                ex = ex_pool.tile([P, HPC, QB], bf16, tag="ex")
                ex_tiles[kc] = ex
                # exp: Act engine evicts head 0 (true Exp), DVE head 1
                # (Schraudolph bit-trick) -- both straight from PSUM
                nc.scalar.activation(
                    ex[:, 0, :],
                    scs[0][:],
                    mybir.ActivationFunctionType.Exp,
                    scale=SCALE,
                )
                with nc.allow_low_precision(reason="schraudolph exp"):
                    nc.vector.tensor_scalar(
                        ex[:, 1, :].bitcast(i16),
                        scs[1][:],
                        EXP_A,
                        EXP_B,
                        mybir.AluOpType.mult,
                        mybir.AluOpType.add,
                    )
                for h in range(HPC):
                    if kc >= LOOK_H[h]:
                        attnv(kc - LOOK_H[h], h)
            for h in range(HPC):
                for kc in range(NKC - LOOK_H[h], NKC):
                    attnv(kc, h)

            for h in range(HPC):
                # denominator row -> f32r sbuf (matmul rhs must be sbuf)
                srow = rc_pool.tile([1, QB], f32r, tag="srow", name=f"sr{qb}_{h}")
                with nc.allow_low_precision(reason="f32r sums, 2^-12 rel"):
                    nc.vector.tensor_copy(srow[:], oT[h][HD : HD + 1, :])
                # K=1 ones-matmul broadcasts the denominators to 64 rows
                rb = rb_pool.tile([HD, QB], f32, tag="rb", name=f"rb{qb}_{h}")
                nc.tensor.matmul(
                    rb[:],
                    lhsT=ones_sb[:, :HD],
                    rhs=srow[:],
                    start=True,
                    stop=True,
                )
                rcb = res_pool.tile([HD, QB], f32, tag="rcb", name=f"rcb{qb}_{h}")
                nc.vector.reciprocal_approx_fast(out=rcb[:], in_=rb[:])
                res = res_pool.tile([HD, QB], bf16, tag="res")
                with nc.allow_low_precision(reason="bf16 output"):
                    nc.vector.tensor_mul(res[:], oT[h][:HD, :], rcb[:])
                nc.sync.dma_start(
                    out=outT[h * HD : (h + 1) * HD, q0:q1], in_=res[:]
                )


def build_nc():
    from contextlib import ExitStack

    nc = bacc.Bacc(
        "TRN2",
        target_bir_lowering=False,
        debug=False,
        num_devices=NCORES,
    )
    xT = nc.dram_tensor("xT", [H, S], bf16, kind="ExternalInput").ap()
    wq = nc.dram_tensor("wq", [H, DPC], bf16, kind="ExternalInput").ap()
    wk = nc.dram_tensor("wk", [H, DPC], bf16, kind="ExternalInput").ap()
    wv = nc.dram_tensor("wv", [H, DPC], bf16, kind="ExternalInput").ap()
    bias3 = nc.dram_tensor("bias3", [3, DPC], f32, kind="ExternalInput").ap()
    ones = nc.dram_tensor("ones", [1, P], f32r, kind="ExternalInput").ap()
    onescol = nc.dram_tensor(
        "onescol", [P, NKC * HPC], bf16, kind="ExternalInput"
    ).ap()
    outT = nc.dram_tensor("outT", [DPC, S], bf16, kind="ExternalOutput").ap()
    with tile.TileContext(nc) as tc, ExitStack() as ctx:
        _emit_kernel(ctx, tc, outT, xT, wq, wk, wv, bias3, ones, onescol)
    nc.compile()
    return nc


_NC_CACHE = None


def _get_nc():
    global _NC_CACHE
    if _NC_CACHE is None:
        _NC_CACHE = build_nc()
    return _NC_CACHE


def _shard_inputs(x, Wq, bq, Wk, bk, Wv, bv):
    """Build per-core input maps (host does layout only: transpose/slice)."""
    x = np.ascontiguousarray(np.asarray(x, dtype=np.float32))
    in_maps = []
    xT_by_batch = [np.ascontiguousarray(x[b].T).astype(_np_bf16) for b in range(B)]
    for c in range(NCORES):
        b, p = c // (NCORES // B), c % (NCORES // B)
        cols = slice(p * DPC, (p + 1) * DPC)
        in_maps.append(
            {
                "xT": xT_by_batch[b],
                "wq": np.asarray(Wq, np.float32)[:, cols].astype(_np_bf16),
                "wk": np.asarray(Wk, np.float32)[:, cols].astype(_np_bf16),
                "wv": np.asarray(Wv, np.float32)[:, cols].astype(_np_bf16),
                "bias3": np.stack(
                    [
                        np.asarray(bq, np.float32)[cols],
                        np.asarray(bk, np.float32)[cols],
                        np.asarray(bv, np.float32)[cols],
                    ]
                ),
                "ones": np.ones((1, P), dtype=np.float32),
                "onescol": np.ones((P, NKC * HPC), dtype=_np_bf16),
            }
        )
    return in_maps


def _assemble(results):
    out = np.empty((B, S, H), dtype=np.float32)
    for c in range(NCORES):
        b, p = c // (NCORES // B), c % (NCORES // B)
        outT = results[c]["outT"]  # [128, S] bf16
        out[b, :, p * DPC : (p + 1) * DPC] = outT.astype(np.float32).T
    return out


def run(inputs, trace=False):
    nc = _get_nc()
    in_maps = _shard_inputs(**inputs)
    res = run_bass_kernel_spmd(nc, in_maps, list(range(NCORES)), trace=trace)
    return _assemble(res.results), res


def kernel(**inputs):
    out, _ = run(inputs)
    return out


# revision 14
# speedup vs baseline: 1.1677x; 1.0047x over previous
"""Trainium2 Bass kernel for an 8-head MultiHeadAttention (B=2, S=4096, H=512).

Sharding: 8 NeuronCores, each takes (one batch, two heads):
    core c -> batch b = c // 4, heads {2*(c%4), 2*(c%4)+1}.

Optimized pipeline (vs. the 358us baseline):
  - Projections run in bf16 (x and W pre-cast on host): halves the input
    DMA (4MB/core) and keeps PE at 1 cycle/row.  Wave order k -> q -> v so
    scores can start as soon as q's first block is evicted.
  - The attention inner loop is software-pipelined with a lookahead of
    LOOK kc-chunks so the PE never waits on the softmax exp:
        scores(kc) -> exp(kc) [split across engines] -> attn@v(kc-LOOK)
  - The exp of each [128, 2*512] score tile is split across THREE engines
    working concurrently on disjoint column ranges: Act (true Exp), DVE
    and GpSimd (Schraudolph bit-trick exp: bf16 bits = int16(s*A + B),
    +-3.3% multiplicative, validated ~1.3e-2 scale-relative absmax).
  - Normalization: denominator row (from the ones-column in v) is copied
    to f32r, broadcast across 64 partitions by a K=1 ones-matmul,
    reciprocal_approx_fast + multiply on DVE, bf16 output (host upcasts).
"""

import sys

sys.path.insert(0, "/opt/trn_rl_repo")

import ml_dtypes
import numpy as np

import concourse.bass as bass  # noqa: E402
import concourse.tile as tile  # noqa: E402
from concourse import bacc, mybir  # noqa: E402
from concourse.bass_utils import run_bass_kernel_spmd  # noqa: E402

B, S, H = 2, 4096, 512
NH, HD = 8, 64
NCORES = 8
HPC = 2  # heads per core
DPC = HPC * HD  # head dims per core = 128
P = 128  # partitions
QB = 512  # query block (matmul free dim)
KC = 128  # key chunk (contraction tile)
KF = H // P  # feature chunks for projections = 4
NKC = S // KC  # 32
NQB = S // QB  # 8
VPAD = 80  # padded per-(kc,h) v row (64 v + ones + align padding)
LOOK = 2  # attn@v lookahead (kc chunks) for PE pipelining

# Per-head exp engines: Act owns head 0, DVE (Schraudolph) owns head 1.
# GpSimd cannot read PSUM, so only these two engines can evict scores.
LOOK_H = (2, 3)  # attn@v lookahead per head (h1 later: DVE exp is slower)

# Schraudolph constants: bf16(exp(s/8)) bits ~= int16(s*EXP_A + EXP_B)
SCALE = 1.0 / np.sqrt(HD)
EXP_A = float(128.0 * np.log2(np.e) * SCALE)
EXP_B = float(128.0 * (127.0 - 0.0436))

f32 = mybir.dt.float32
f32r = mybir.dt.float32r
bf16 = mybir.dt.bfloat16
i16 = mybir.dt.int16
_np_bf16 = ml_dtypes.bfloat16


def _emit_kernel(ctx, tc, outT, xT, wq, wk, wv, bias3, ones, onescol):
    nc = tc.nc

    const = ctx.enter_context(tc.tile_pool(name="const", bufs=1))

    # ---- DMA order: xT chunk 0 first (the k-wave gates on it), then
    # weights, remaining chunks interleaved.  Weights come pre-arranged
    # from the host as [128, KF*128] so each DMA is contiguous/partition.
    wq_sb = const.tile([P, KF, DPC], bf16)
    wk_sb = const.tile([P, KF, DPC], bf16)
    wv_sb = const.tile([P, KF, DPC], bf16)
    bias_sb = const.tile([P, 3], f32)
    xT_sb = const.tile([P, KF, S], bf16)

    def xchunk(kf, hh):
        nc.sync.dma_start(
            out=xT_sb[:, kf, hh * (S // 2) : (hh + 1) * (S // 2)],
            in_=xT[kf * P : (kf + 1) * P, hh * (S // 2) : (hh + 1) * (S // 2)],
        )

    xchunk(0, 0)
    xchunk(0, 1)
    nc.sync.dma_start(out=wk_sb.rearrange("p a b -> p (a b)"), in_=wk[:])
    # biases [3, 128] -> sbuf [128, 3] (partition = output dim; q, k, v)
    nc.sync.dma_start(out=bias_sb[:], in_=bias3.rearrange("a m -> m a"))
    xchunk(1, 0)
    xchunk(1, 1)
    nc.sync.dma_start(out=wq_sb.rearrange("p a b -> p (a b)"), in_=wq[:])
    xchunk(2, 0)
    xchunk(2, 1)
    nc.sync.dma_start(out=wv_sb.rearrange("p a b -> p (a b)"), in_=wv[:])
    xchunk(3, 0)
    xchunk(3, 1)

    # ---- projections: q/k/v in T layout, bf16 matmuls, bf16 evictions ----
    qkT_sb = const.tile([P, 2, S], bf16)
    vT_sb = const.tile([P, S], bf16)
    # v natural + ones column: vp_sb[p, kc, h, :64] = v, [..., 64] = 1
    vp_sb = const.tile([P, NKC, HPC, VPAD], bf16)
    ones_sb = const.tile([1, P], f32r)
    nc.sync.dma_start(out=ones_sb[:], in_=ones[:])
    nc.sync.dma_start(out=vp_sb[:, :, :, HD : HD + 1], in_=onescol[:])

    with tc.tile_pool(name="proj_psum", bufs=8, space="PSUM") as pp:
        with nc.named_scope("proj"):
            # wave order: k, q (so scores can start), then v
            for proj, w_sb in ((1, wk_sb), (0, wq_sb), (2, wv_sb)):
                pss = [
                    pp.tile([P, QB], f32, tag="ps", name=f"pj{proj}_{sb}")
                    for sb in range(S // QB)
                ]
                # kf-outer: the first 8 matmuls need only xT chunk 0
                for kf in range(KF):
                    for sb in range(S // QB):
                        nc.tensor.matmul(
                            pss[sb][:],
                            lhsT=w_sb[:, kf, :],
                            rhs=xT_sb[:, kf, sb * QB : (sb + 1) * QB],
                            start=(kf == 0),
                            stop=(kf == KF - 1),
                        )
                for sb in range(S // QB):
                    dst = (
                        vT_sb[:, sb * QB : (sb + 1) * QB]
                        if proj == 2
                        else qkT_sb[:, proj, sb * QB : (sb + 1) * QB]
                    )
                    # psum -> sbuf eviction, fused bias add, bf16 out
                    with nc.allow_low_precision(reason="bf16 attention"):
                        nc.vector.tensor_scalar_add(
                            dst, pss[sb][:], bias_sb[:, proj : proj + 1]
                        )
                if proj == 2:
                    # v: T layout -> natural via hardware DMA transpose
                    # (X-bar, bf16), one per head: in [64, S] -> out
                    # [128, NKC, 64].  All input DMAs have drained by now;
                    # the transposes overlap the first scores/exp waves.
                    # (Finer-grained splits that overlap the eviction
                    # stream hard-crash the device - do not pipeline these.)
                    for h in range(HPC):
                        nc.sync.dma_start_transpose(
                            out=vp_sb[:, :, h, 0:HD],
                            in_=vT_sb[h * HD : (h + 1) * HD, :],
                        )

    # ---- attention ----
    # PSUM budget (8 banks): sch 4 + oT 2 + rb 2
    sc_pool = ctx.enter_context(tc.tile_pool(name="sc", bufs=4, space="PSUM"))
    rb_pool = ctx.enter_context(tc.tile_pool(name="rb", bufs=2, space="PSUM"))
    ot_pool = ctx.enter_context(tc.tile_pool(name="ot", bufs=1, space="PSUM"))
    ex_pool = ctx.enter_context(tc.tile_pool(name="ex", bufs=5))
    rc_pool = ctx.enter_context(tc.tile_pool(name="rc", bufs=4))
    res_pool = ctx.enter_context(tc.tile_pool(name="res", bufs=4))

    with nc.named_scope("attn"):
        for qb in range(NQB):
            q0, q1 = qb * QB, (qb + 1) * QB
            oT = [
                ot_pool.tile([HD + 1, QB], f32, tag=f"oT{h}", name=f"oT{qb}_{h}")
                for h in range(HPC)
            ]
            ex_tiles = {}

            def attnv(kc, h):
                nc.tensor.matmul(
                    oT[h][:],
                    lhsT=vp_sb[:, kc, h, 0 : HD + 1],
                    rhs=ex_tiles[kc][h][:],
                    start=(kc == 0),
                    stop=(kc == NKC - 1),
                )

            for kc in range(NKC):
                # per-head 1-bank score tiles -> deeper recycle slack
                scs = [
                    sc_pool.tile([P, QB], f32, tag="sch", name=f"sc{qb}_{kc}_{h}")
                    for h in range(HPC)
                ]
                for h in range(HPC):
                    # scoresT[k, q] for head h; K = 64, rows 64h..64h+63
                    nc.tensor.matmul(
                        scs[h][:],
                        lhsT=qkT_sb[
                            h * HD : (h + 1) * HD, 1, kc * KC : (kc + 1) * KC
                        ],
                        rhs=qkT_sb[h * HD : (h + 1) * HD, 0, q0:q1],
                        start=True,
                        stop=True,
                        tile_position=(h * HD, 0),
                    )
                # separate per-head ex tiles: a shared tile would create a
                # false WAW between the two engines' writes
                exA = ex_pool.tile([P, QB], bf16, tag="exA", name=f"exA{qb}_{kc}")
                exB = ex_pool.tile([P, QB], bf16, tag="exB", name=f"exB{qb}_{kc}")
                ex_tiles[kc] = (exA, exB)
                # exp: Act engine evicts head 0 (true Exp), DVE head 1
                # (Schraudolph bit-trick) -- both straight from PSUM
                nc.scalar.activation(
                    exA[:],
                    scs[0][:],
                    mybir.ActivationFunctionType.Exp,
                    scale=SCALE,
                )
                with nc.allow_low_precision(reason="schraudolph exp"):
                    nc.vector.tensor_scalar(
                        exB[:].bitcast(i16),
                        scs[1][:],
                        EXP_A,
                        EXP_B,
                        mybir.AluOpType.mult,
                        mybir.AluOpType.add,
                    )
                for h in range(HPC):
                    if kc >= LOOK_H[h]:
                        attnv(kc - LOOK_H[h], h)
            for h in range(HPC):
                for kc in range(NKC - LOOK_H[h], NKC):
                    attnv(kc, h)

            for h in range(HPC):
                # denominator row -> f32r sbuf (matmul rhs must be sbuf)
                srow = rc_pool.tile([1, QB], f32r, tag="srow", name=f"sr{qb}_{h}")
                with nc.allow_low_precision(reason="f32r sums, 2^-12 rel"):
                    nc.vector.tensor_copy(srow[:], oT[h][HD : HD + 1, :])
                # K=1 ones-matmul broadcasts the denominators to 64 rows
                rb = rb_pool.tile([HD, QB], f32, tag="rb", name=f"rb{qb}_{h}")
                nc.tensor.matmul(
                    rb[:],
                    lhsT=ones_sb[:, :HD],
                    rhs=srow[:],
                    start=True,
                    stop=True,
                )
                rcb = res_pool.tile([HD, QB], f32, tag="rcb", name=f"rcb{qb}_{h}")
                nc.vector.reciprocal_approx_fast(out=rcb[:], in_=rb[:])
                res = res_pool.tile([HD, QB], bf16, tag="res")
                with nc.allow_low_precision(reason="bf16 output"):
                    nc.vector.tensor_mul(res[:], oT[h][:HD, :], rcb[:])
                nc.sync.dma_start(
                    out=outT[h * HD : (h + 1) * HD, q0:q1], in_=res[:]
                )


def build_nc():
    from contextlib import ExitStack

    nc = bacc.Bacc(
        "TRN2",
        target_bir_lowering=False,
        debug=False,
        num_devices=NCORES,
    )
    xT = nc.dram_tensor("xT", [H, S], bf16, kind="ExternalInput").ap()
    # weights pre-arranged on host to [128, KF*128] (partition-contiguous)
    wq = nc.dram_tensor("wq", [P, KF * DPC], bf16, kind="ExternalInput").ap()
    wk = nc.dram_tensor("wk", [P, KF * DPC], bf16, kind="ExternalInput").ap()
    wv = nc.dram_tensor("wv", [P, KF * DPC], bf16, kind="ExternalInput").ap()
    bias3 = nc.dram_tensor("bias3", [3, DPC], f32, kind="ExternalInput").ap()
    ones = nc.dram_tensor("ones", [1, P], f32r, kind="ExternalInput").ap()
    onescol = nc.dram_tensor(
        "onescol", [P, NKC * HPC], bf16, kind="ExternalInput"
    ).ap()
    outT = nc.dram_tensor("outT", [DPC, S], bf16, kind="ExternalOutput").ap()
    with tile.TileContext(nc) as tc, ExitStack() as ctx:
        _emit_kernel(ctx, tc, outT, xT, wq, wk, wv, bias3, ones, onescol)
    nc.compile()
    return nc


_NC_CACHE = None


def _get_nc():
    global _NC_CACHE
    if _NC_CACHE is None:
        _NC_CACHE = build_nc()
    return _NC_CACHE


def _shard_inputs(x, Wq, bq, Wk, bk, Wv, bv):
    """Build per-core input maps (host does layout only: transpose/slice)."""
    x = np.ascontiguousarray(np.asarray(x, dtype=np.float32))
    in_maps = []
    xT_by_batch = [np.ascontiguousarray(x[b].T).astype(_np_bf16) for b in range(B)]

    def warr(W, cols):
        # [512, 128] -> [128 (p), KF*128] so the device DMA is contiguous
        w = np.asarray(W, np.float32)[:, cols].astype(_np_bf16)
        return np.ascontiguousarray(
            w.reshape(KF, P, DPC).transpose(1, 0, 2).reshape(P, KF * DPC)
        )

    for c in range(NCORES):
        b, p = c // (NCORES // B), c % (NCORES // B)
        cols = slice(p * DPC, (p + 1) * DPC)
        in_maps.append(
            {
                "xT": xT_by_batch[b],
                "wq": warr(Wq, cols),
                "wk": warr(Wk, cols),
                "wv": warr(Wv, cols),
                "ones": np.ones((1, P), dtype=np.float32),
                "onescol": np.ones((P, NKC * HPC), dtype=_np_bf16),
                "bias3": np.stack(
                    [
                        np.asarray(bq, np.float32)[cols],
                        np.asarray(bk, np.float32)[cols],
                        np.asarray(bv, np.float32)[cols],
                    ]
                ),
            }
        )
    return in_maps


def _assemble(results):
    out = np.empty((B, S, H), dtype=np.float32)
    for c in range(NCORES):
        b, p = c // (NCORES // B), c % (NCORES // B)
        outT = results[c]["outT"]  # [128, S] bf16
        out[b, :, p * DPC : (p + 1) * DPC] = outT.astype(np.float32).T
    return out


def run(inputs, trace=False):
    nc = _get_nc()
    in_maps = _shard_inputs(**inputs)
    res = run_bass_kernel_spmd(nc, in_maps, list(range(NCORES)), trace=trace)
    return _assemble(res.results), res


def kernel(**inputs):
    out, _ = run(inputs)
    return out


# revision 18
# speedup vs baseline: 1.2510x; 1.0713x over previous
"""Trainium2 Bass kernel for an 8-head MultiHeadAttention (B=2, S=4096, H=512).

Sharding: 8 NeuronCores, each takes (one batch, two heads):
    core c -> batch b = c // 4, heads {2*(c%4), 2*(c%4)+1}.

Optimized pipeline (vs. the 358us baseline):
  - Projections run in bf16 (x and W pre-cast on host): halves the input
    DMA (4MB/core) and keeps PE at 1 cycle/row.  Wave order k -> q -> v so
    scores can start as soon as q's first block is evicted.
  - The attention inner loop is software-pipelined with a lookahead of
    LOOK kc-chunks so the PE never waits on the softmax exp:
        scores(kc) -> exp(kc) [split across engines] -> attn@v(kc-LOOK)
  - The exp of each [128, 2*512] score tile is split across THREE engines
    working concurrently on disjoint column ranges: Act (true Exp), DVE
    and GpSimd (Schraudolph bit-trick exp: bf16 bits = int16(s*A + B),
    +-3.3% multiplicative, validated ~1.3e-2 scale-relative absmax).
  - Normalization: denominator row (from the ones-column in v) is copied
    to f32r, broadcast across 64 partitions by a K=1 ones-matmul,
    reciprocal_approx_fast + multiply on DVE, bf16 output (host upcasts).
"""

import sys

sys.path.insert(0, "/opt/trn_rl_repo")

import ml_dtypes
import numpy as np

import concourse.bass as bass  # noqa: E402
import concourse.tile as tile  # noqa: E402
from concourse import bacc, mybir  # noqa: E402
from concourse.bass_utils import run_bass_kernel_spmd  # noqa: E402

B, S, H = 2, 4096, 512
NH, HD = 8, 64
NCORES = 8
HPC = 2  # heads per core
DPC = HPC * HD  # head dims per core = 128
P = 128  # partitions
QB = 512  # query block (matmul free dim)
KC = 128  # key chunk (contraction tile)
KF = H // P  # feature chunks for projections = 4
NKC = S // KC  # 32
NQB = S // QB  # 8
VPAD = 80  # padded per-(kc,h) v row (64 v + ones + align padding)
LOOK = 2  # attn@v lookahead (kc chunks) for PE pipelining

# Per-head exp engines: Act owns head 0, DVE (Schraudolph) owns head 1.
# GpSimd cannot read PSUM, so only these two engines can evict scores.
LOOK_H = (2, 3)  # attn@v lookahead per head (h1 later: DVE exp is slower)

# Schraudolph constants: bf16(exp(s/8)) bits ~= int16(s*EXP_A + EXP_B)
SCALE = 1.0 / np.sqrt(HD)
EXP_A = float(128.0 * np.log2(np.e) * SCALE)
EXP_B = float(128.0 * (127.0 - 0.0436))

f32 = mybir.dt.float32
f32r = mybir.dt.float32r
bf16 = mybir.dt.bfloat16
i16 = mybir.dt.int16
_np_bf16 = ml_dtypes.bfloat16


def _emit_kernel(ctx, tc, outT, xT, wq, wk, wv, bias3, ones, onescol):
    nc = tc.nc

    const = ctx.enter_context(tc.tile_pool(name="const", bufs=1))

    # ---- DMA order: xT chunk 0 first (the k-wave gates on it), then
    # weights, remaining chunks interleaved.  Weights come pre-arranged
    # from the host as [128, KF*128] so each DMA is contiguous/partition.
    wq_sb = const.tile([P, KF, DPC], bf16)
    wk_sb = const.tile([P, KF, DPC], bf16)
    wv_sb = const.tile([P, KF, DPC], bf16)
    bias_sb = const.tile([P, 3], f32)
    xT_sb = const.tile([P, KF, S], bf16)

    def xchunk(kf, hh):
        nc.sync.dma_start(
            out=xT_sb[:, kf, hh * (S // 2) : (hh + 1) * (S // 2)],
            in_=xT[kf * P : (kf + 1) * P, hh * (S // 2) : (hh + 1) * (S // 2)],
        )

    xchunk(0, 0)
    xchunk(0, 1)
    nc.sync.dma_start(out=wk_sb.rearrange("p a b -> p (a b)"), in_=wk[:])
    # biases [3, 128] -> sbuf [128, 3] (partition = output dim; q, k, v)
    nc.sync.dma_start(out=bias_sb[:], in_=bias3.rearrange("a m -> m a"))
    xchunk(1, 0)
    xchunk(1, 1)
    nc.sync.dma_start(out=wq_sb.rearrange("p a b -> p (a b)"), in_=wq[:])
    xchunk(2, 0)
    xchunk(2, 1)
    nc.sync.dma_start(out=wv_sb.rearrange("p a b -> p (a b)"), in_=wv[:])
    xchunk(3, 0)
    xchunk(3, 1)

    # ---- projections: q/k/v in T layout, bf16 matmuls, bf16 evictions ----
    # k is stored zero-PADDED to K=128 per head (kp_sb[:, h]: head h's
    # 64 dims on its own partition range, zeros elsewhere) so the score
    # matmuls run in the same (128, 128) PE tiling mode as attn@v --
    # avoiding a TensorE drain on every mode switch.
    qT_sb = const.tile([P, S], bf16)
    kp_sb = const.tile([P, 2, S], bf16)
    nc.vector.memset(kp_sb[HD:P, 0, :], 0.0)
    nc.vector.memset(kp_sb[0:HD, 1, :], 0.0)
    vT_sb = const.tile([P, S], bf16)
    # v natural + ones column: vp_sb[p, kc, h, :64] = v, [..., 64] = 1
    vp_sb = const.tile([P, NKC, HPC, VPAD], bf16)
    ones_sb = const.tile([1, P], f32r)
    nc.sync.dma_start(out=ones_sb[:], in_=ones[:])
    nc.sync.dma_start(out=vp_sb[:, :, :, HD : HD + 1], in_=onescol[:])

    with tc.tile_pool(name="proj_psum", bufs=8, space="PSUM") as pp:
        with nc.named_scope("proj"):
            # wave order: k, q (so scores can start), then v
            for proj, w_sb in ((1, wk_sb), (0, wq_sb), (2, wv_sb)):
                pss = [
                    pp.tile([P, QB], f32, tag="ps", name=f"pj{proj}_{sb}")
                    for sb in range(S // QB)
                ]
                # kf-outer: the first 8 matmuls need only xT chunk 0
                for kf in range(KF):
                    for sb in range(S // QB):
                        nc.tensor.matmul(
                            pss[sb][:],
                            lhsT=w_sb[:, kf, :],
                            rhs=xT_sb[:, kf, sb * QB : (sb + 1) * QB],
                            start=(kf == 0),
                            stop=(kf == KF - 1),
                        )
                for sb in range(S // QB):
                    s0, s1 = sb * QB, (sb + 1) * QB
                    # psum -> sbuf eviction, fused bias add, bf16 out
                    with nc.allow_low_precision(reason="bf16 attention"):
                        if proj == 1:  # k: two per-head padded evictions
                            for h in range(HPC):
                                rows = slice(h * HD, (h + 1) * HD)
                                nc.vector.tensor_scalar_add(
                                    kp_sb[rows, h, s0:s1],
                                    pss[sb][rows, :],
                                    bias_sb[rows, proj : proj + 1],
                                )
                        else:
                            dst = (
                                vT_sb[:, s0:s1]
                                if proj == 2
                                else qT_sb[:, s0:s1]
                            )
                            nc.vector.tensor_scalar_add(
                                dst, pss[sb][:], bias_sb[:, proj : proj + 1]
                            )
                if proj == 2:
                    # v: T layout -> natural via hardware DMA transpose
                    # (X-bar, bf16), one per head: in [64, S] -> out
                    # [128, NKC, 64].  All input DMAs have drained by now;
                    # the transposes overlap the first scores/exp waves.
                    # (Finer-grained splits that overlap the eviction
                    # stream hard-crash the device - do not pipeline these.)
                    for h in range(HPC):
                        nc.sync.dma_start_transpose(
                            out=vp_sb[:, :, h, 0:HD],
                            in_=vT_sb[h * HD : (h + 1) * HD, :],
                        )

    # ---- attention ----
    # PSUM budget (8 banks): sch 6 + oT 2
    sc_pool = ctx.enter_context(tc.tile_pool(name="sc", bufs=6, space="PSUM"))
    ot_pool = ctx.enter_context(tc.tile_pool(name="ot", bufs=1, space="PSUM"))
    ex_pool = ctx.enter_context(tc.tile_pool(name="ex", bufs=5))
    rc_pool = ctx.enter_context(tc.tile_pool(name="rc", bufs=4))
    res_pool = ctx.enter_context(tc.tile_pool(name="res", bufs=4))

    with nc.named_scope("attn"):
        for qb in range(NQB):
            q0, q1 = qb * QB, (qb + 1) * QB
            oT = [
                ot_pool.tile([HD + 1, QB], f32, tag=f"oT{h}", name=f"oT{qb}_{h}")
                for h in range(HPC)
            ]
            ex_tiles = {}

            def attnv(kc, h):
                nc.tensor.matmul(
                    oT[h][:],
                    lhsT=vp_sb[:, kc, h, 0 : HD + 1],
                    rhs=ex_tiles[kc][h][:],
                    start=(kc == 0),
                    stop=(kc == NKC - 1),
                )

            for kc in range(NKC):
                # per-head 1-bank score tiles -> deeper recycle slack
                scs = [
                    sc_pool.tile([P, QB], f32, tag="sch", name=f"sc{qb}_{kc}_{h}")
                    for h in range(HPC)
                ]
                for h in range(HPC):
                    # scoresT[k, q] for head h; K = 128 via the zero-padded
                    # k tile -> same PE tiling mode as attn@v (no drains)
                    nc.tensor.matmul(
                        scs[h][:],
                        lhsT=kp_sb[:, h, kc * KC : (kc + 1) * KC],
                        rhs=qT_sb[:, q0:q1],
                        start=True,
                        stop=True,
                    )
                # separate per-head ex tiles: a shared tile would create a
                # false WAW between the two engines' writes
                exA = ex_pool.tile([P, QB], bf16, tag="exA", name=f"exA{qb}_{kc}")
                exB = ex_pool.tile([P, QB], bf16, tag="exB", name=f"exB{qb}_{kc}")
                ex_tiles[kc] = (exA, exB)
                # exp: Act engine evicts head 0 (true Exp), DVE head 1
                # (Schraudolph bit-trick) -- both straight from PSUM
                nc.scalar.activation(
                    exA[:],
                    scs[0][:],
                    mybir.ActivationFunctionType.Exp,
                    scale=SCALE,
                )
                with nc.allow_low_precision(reason="schraudolph exp"):
                    nc.vector.tensor_scalar(
                        exB[:].bitcast(i16),
                        scs[1][:],
                        EXP_A,
                        EXP_B,
                        mybir.AluOpType.mult,
                        mybir.AluOpType.add,
                    )
                for h in range(HPC):
                    if kc >= LOOK_H[h]:
                        attnv(kc - LOOK_H[h], h)
            for h in range(HPC):
                for kc in range(NKC - LOOK_H[h], NKC):
                    attnv(kc, h)

            for h in range(HPC):
                # denominator row -> sbuf, reciprocal, then broadcast to 64
                # partitions on GpSimd (keeps the PE out of normalization)
                srow = rc_pool.tile([1, QB], f32, tag="srow", name=f"sr{qb}_{h}")
                nc.vector.tensor_copy(srow[:], oT[h][HD : HD + 1, :])
                rsr = rc_pool.tile([1, QB], f32, tag="rsr", name=f"rs{qb}_{h}")
                nc.vector.reciprocal_approx_fast(out=rsr[:], in_=srow[:])
                rcb = res_pool.tile([HD, QB], f32, tag="rcb", name=f"rcb{qb}_{h}")
                nc.gpsimd.partition_broadcast(rcb[:], rsr[:])
                res = res_pool.tile([HD, QB], bf16, tag="res")
                with nc.allow_low_precision(reason="bf16 output"):
                    nc.vector.tensor_mul(res[:], oT[h][:HD, :], rcb[:])
                nc.sync.dma_start(
                    out=outT[h * HD : (h + 1) * HD, q0:q1], in_=res[:]
                )


def build_nc():
    from contextlib import ExitStack

    nc = bacc.Bacc(
        "TRN2",
        target_bir_lowering=False,
        debug=False,
        num_devices=NCORES,
    )
    xT = nc.dram_tensor("xT", [H, S], bf16, kind="ExternalInput").ap()
    # weights pre-arranged on host to [128, KF*128] (partition-contiguous)
    wq = nc.dram_tensor("wq", [P, KF * DPC], bf16, kind="ExternalInput").ap()
    wk = nc.dram_tensor("wk", [P, KF * DPC], bf16, kind="ExternalInput").ap()
    wv = nc.dram_tensor("wv", [P, KF * DPC], bf16, kind="ExternalInput").ap()
    bias3 = nc.dram_tensor("bias3", [3, DPC], f32, kind="ExternalInput").ap()
    ones = nc.dram_tensor("ones", [1, P], f32r, kind="ExternalInput").ap()
    onescol = nc.dram_tensor(
        "onescol", [P, NKC * HPC], bf16, kind="ExternalInput"
    ).ap()
    outT = nc.dram_tensor("outT", [DPC, S], bf16, kind="ExternalOutput").ap()
    with tile.TileContext(nc) as tc, ExitStack() as ctx:
        _emit_kernel(ctx, tc, outT, xT, wq, wk, wv, bias3, ones, onescol)
    nc.compile()
    return nc


_NC_CACHE = None


def _get_nc():
    global _NC_CACHE
    if _NC_CACHE is None:
        _NC_CACHE = build_nc()
    return _NC_CACHE


def _shard_inputs(x, Wq, bq, Wk, bk, Wv, bv):
    """Build per-core input maps (host does layout only: transpose/slice)."""
    x = np.ascontiguousarray(np.asarray(x, dtype=np.float32))
    in_maps = []
    xT_by_batch = [np.ascontiguousarray(x[b].T).astype(_np_bf16) for b in range(B)]

    def warr(W, cols):
        # [512, 128] -> [128 (p), KF*128] so the device DMA is contiguous
        w = np.asarray(W, np.float32)[:, cols].astype(_np_bf16)
        return np.ascontiguousarray(
            w.reshape(KF, P, DPC).transpose(1, 0, 2).reshape(P, KF * DPC)
        )

    for c in range(NCORES):
        b, p = c // (NCORES // B), c % (NCORES // B)
        cols = slice(p * DPC, (p + 1) * DPC)
        in_maps.append(
            {
                "xT": xT_by_batch[b],
                "wq": warr(Wq, cols),
                "wk": warr(Wk, cols),
                "wv": warr(Wv, cols),
                "ones": np.ones((1, P), dtype=np.float32),
                "onescol": np.ones((P, NKC * HPC), dtype=_np_bf16),
                "bias3": np.stack(
                    [
                        np.asarray(bq, np.float32)[cols],
                        np.asarray(bk, np.float32)[cols],
                        np.asarray(bv, np.float32)[cols],
                    ]
                ),
            }
        )
    return in_maps


def _assemble(results):
    out = np.empty((B, S, H), dtype=np.float32)
    for c in range(NCORES):
        b, p = c // (NCORES // B), c % (NCORES // B)
        outT = results[c]["outT"]  # [128, S] bf16
        out[b, :, p * DPC : (p + 1) * DPC] = outT.astype(np.float32).T
    return out


def run(inputs, trace=False):
    nc = _get_nc()
    in_maps = _shard_inputs(**inputs)
    res = run_bass_kernel_spmd(nc, in_maps, list(range(NCORES)), trace=trace)
    return _assemble(res.results), res


def kernel(**inputs):
    out, _ = run(inputs)
    return out
